# revision 1
# baseline (speedup 1.0000x reference)
"""DimGraphConv (GNN mean-aggregation message passing) on 8 Trainium2 cores.

Math (per reference):
    out = x @ W_self.T + segment_mean(x[row], col) @ W_neigh.T + bias

Because the neighbor transform is linear, we aggregate raw features first
(segment-sum + in-degree on device), then apply one small GEMM per 128-node
slot:  out^T = W_neigh @ (agg/deg)^T + W_self @ x^T + bias.

Sharding ("scatter to node owners" variant of the hint): edges are
partitioned on the host by TARGET owner -- core c owns nodes
[c*12500, (c+1)*12500) -- so no cross-core collective is needed. x is
replicated; each core dma_gathers its edges' source rows (256B each) from
HBM and accumulates them into an SBUF-resident accumulator with the SDMA
CCE scatter-add (dma_scatter_add, parity-split SBUF destination mode,
tokens_per_rank=128). A second elem16 scatter of ones accumulates the
in-degree.

Hardware constraints handled on the host (pure edge reordering/padding --
the sums themselves all happen on device):
  * gather indices are int16 -> edges are grouped by source bank
    (4 banks x 25000 rows); each gather instruction reads one bank.
  * CCE read-modify-write races when two tokens in the SAME scatter
    instruction hit the same cell -> within each chunk all targets are
    distinct. Edges of one (core, bank) are dealt into NCHB chunks so that
    equal-target edges land in different chunks (offset round-robin).
  * idx tiles must hold the [16, n/16]-wrapped pattern replicated across
    all 8 16-partition groups (different Q7 cores read different copies).
  * pad tokens gather row 0 and scatter into a dead cell (node id 12500),
    so every token is valid and the SPMD program is identical on all cores.
"""
import sys

sys.path.insert(0, "/opt/trn_rl_repo")

import numpy as np


# ---------------------------------------------------------------- config
class _Cfg:
    N = 100000          # nodes
    D = 64              # feature dim (256B rows = dma_gather elem)
    NCORE = 8
    W = 12500           # target window per core
    NBANK = 4           # source banks (int16 gather index limit)
    BANK = 25000        # rows per source bank
    M = 2816            # tokens per chunk (22*128)
    NCHB = 20           # chunks per (core, bank); max per-bucket target
                        # multiplicity must stay < NCHB (Poisson(4) here)
    EW = 68             # scatter payload elems: 64 features + degree + pad
    REPEAT = 1          # timing-only: re-run the phase-1 loop this many times
    GRP = 7             # phase-2 slots per output DMA group

    @property
    def BANK_CAP(self):
        return self.M * self.NCHB

    @property
    def NCH(self):
        return self.NBANK * self.NCHB

    @property
    def TOK(self):
        return self.NBANK * self.BANK_CAP

    @property
    def NSLOT(self):
        # slots of 128 target nodes; last slot also holds the dead pad cell
        return self.W // 128 + 1

    @property
    def NGRP(self):
        return (self.NSLOT + 1) // 2

    @property
    def WPAD(self):
        return self.NSLOT * 128

    @property
    def CPC(self):
        return self.M // 16


CFG = _Cfg()


def _build(cfg):
    """Build + compile the SPMD Bass program. Returns the Bacc instance."""
    import concourse.tile as tile
    from concourse import bacc, mybir
    from concourse.masks import make_identity

    P = 128
    D = cfg.D
    MS = cfg.M // P              # gather tile free rows
    f32 = mybir.dt.float32

    nc = bacc.Bacc(None, target_bir_lowering=False, debug=False)
    x_d = nc.dram_tensor("x", [cfg.N, D], f32, kind="ExternalInput")
    xT_d = nc.dram_tensor("xT", [D, cfg.WPAD], f32, kind="ExternalInput")
    gidx_d = nc.dram_tensor("gidx", [128, cfg.TOK // 16], mybir.dt.int16,
                            kind="ExternalInput")
    sidx_d = nc.dram_tensor("sidx", [128, cfg.TOK // 16], mybir.dt.int16,
                            kind="ExternalInput")
    wnT_d = nc.dram_tensor("wnT", [D, D], f32, kind="ExternalInput")
    wsT_d = nc.dram_tensor("wsT", [D, D], f32, kind="ExternalInput")
    bias_d = nc.dram_tensor("bias", [D, 1], f32, kind="ExternalInput")
    outT_d = nc.dram_tensor("outT", [D, cfg.WPAD], f32, kind="ExternalOutput")

    with tile.TileContext(nc) as tc:
        with (
            tc.tile_pool(name="acc", bufs=1) as accp,
            tc.tile_pool(name="io", bufs=4) as iop,
            tc.tile_pool(name="p2", bufs=3) as p2p,
            tc.tile_pool(name="psum", bufs=2, space="PSUM") as psp,
        ):
            gidx_t = accp.tile([128, cfg.TOK // 16], mybir.dt.int16)
            sidx_t = accp.tile([128, cfg.TOK // 16], mybir.dt.int16)
            nc.sync.dma_start(gidx_t[:], gidx_d[:])
            nc.sync.dma_start(sidx_t[:], sidx_d[:])

            EW = cfg.EW
            own_t = accp.tile([P, cfg.NGRP * EW], f32)
            peer_t = accp.tile([P, cfg.NGRP * EW], f32)
            nc.vector.memset(own_t[:], 0.0)
            nc.vector.memset(peer_t[:], 0.0)

            # ---- phase 1: gather sources + scatter-add into SBUF acc
            for _rep in range(cfg.REPEAT):
              for ch in range(cfg.NCH):
                  bank = ch // cfg.NCHB
                  sl = slice(ch * cfg.CPC, (ch + 1) * cfg.CPC)
                  gath_t = iop.tile([P, MS * D], f32)
                  nc.gpsimd.dma_gather(
                      out_ap=gath_t[:].rearrange("p (m e) -> p m e", e=D),
                      in_ap=x_d[bank * cfg.BANK:(bank + 1) * cfg.BANK, :],
                      idxs_ap=gidx_t[:, sl],
                      num_idxs=cfg.M,
                      num_idxs_reg=cfg.M,
                      elem_size=D,
                      single_packet=False,
                  )
                  # widen each token row to 68 f32: cols 0..63 features,
                  # col 64 = 1.0 (degree), cols 65..67 unused filler
                  aug_t = iop.tile([P, MS * EW], f32)
                  aug3 = aug_t[:].rearrange("p (m e) -> p m e", e=EW)
                  nc.vector.memset(aug3[:, :, D:EW], 1.0)
                  nc.vector.tensor_copy(
                      aug3[:, :, 0:D],
                      gath_t[:].rearrange("p (m e) -> p m e", e=D))
                  nc.gpsimd.dma_scatter_add(
                      out_ap=own_t[:],
                      in_ap=aug3,
                      idxs_ap=sidx_t[:, sl],
                      num_idxs=cfg.M,
                      num_idxs_reg=cfg.M,
                      elem_size=EW,
                      out_ap_other=peer_t[:],
                      parity_reg=0,
                      sbuf_tokens_per_rank=128,
                      single_packet=False,
                  )

            # ---- phase 2: mean, two GEMMs, bias; output transposed
            ident_t = p2p.tile([P, P], f32)
            make_identity(nc, ident_t[:])
            wnT_t = p2p.tile([D, D], f32)
            wsT_t = p2p.tile([D, D], f32)
            bias_t = p2p.tile([D, 1], f32)
            nc.sync.dma_start(wnT_t[:], wnT_d[:])
            nc.sync.dma_start(wsT_t[:], wsT_d[:])
            nc.sync.dma_start(bias_t[:], bias_d[:])

            ngroups_out = cfg.NSLOT // cfg.GRP
            assert ngroups_out * cfg.GRP == cfg.NSLOT
            gw = cfg.GRP * P
            for og in range(ngroups_out):
                xTg_t = p2p.tile([D, gw], f32)
                outg_t = p2p.tile([D, gw], f32)
                nc.sync.dma_start(xTg_t[:], xT_d[:, og * gw:(og + 1) * gw])
                for k in range(cfg.GRP):
                    s = og * cfg.GRP + k
                    g = s >> 1
                    accb = own_t if (s & 1) == 0 else peer_t
                    acc_sl = accb[:, g * EW:g * EW + D]
                    deg_sl = accb[:, g * EW + D:g * EW + D + 1]
                    degc_t = p2p.tile([P, 1], f32)
                    recip_t = p2p.tile([P, 1], f32)
                    mean_t = p2p.tile([P, D], f32)
                    nc.vector.tensor_scalar_max(degc_t[:], deg_sl, 1.0)
                    nc.vector.reciprocal(recip_t[:], degc_t[:])
                    nc.vector.tensor_scalar_mul(mean_t[:], acc_sl,
                                                recip_t[:, 0:1])
                    psT_t = psp.tile([D, P], f32)
                    nc.tensor.transpose(psT_t[:], mean_t[:], ident_t[:])
                    meanT_t = p2p.tile([D, P], f32)
                    nc.vector.tensor_copy(meanT_t[:], psT_t[:])
                    po_t = psp.tile([D, P], f32)
                    nc.tensor.matmul(po_t[:], lhsT=wnT_t[:], rhs=meanT_t[:],
                                     start=True, stop=False)
                    nc.tensor.matmul(po_t[:], lhsT=wsT_t[:],
                                     rhs=xTg_t[:, k * P:(k + 1) * P],
                                     start=False, stop=True)
                    nc.vector.tensor_scalar_add(outg_t[:, k * P:(k + 1) * P],
                                                po_t[:], bias_t[:, 0:1])
                nc.sync.dma_start(outT_d[:, og * gw:(og + 1) * gw], outg_t[:])

    nc.compile()
    return nc


def _pack_bucket(t_local, lo, nchb, m):
    """Deal one (core, bank) bucket's edges into nchb chunks of capacity m
    such that within a chunk all targets are distinct. Returns (chunk_id,
    pos_in_chunk) per edge, aligned with the input order.

    Equal-target occurrences get chunk (occ_idx + hash(t)) % nchb, which is
    injective per target as long as multiplicity <= nchb. A few hash seeds
    are tried if a chunk overflows its capacity."""
    n = len(t_local)
    if n == 0:
        return np.zeros(0, np.int64), np.zeros(0, np.int64)
    order = np.argsort(t_local, kind="stable")
    ts = t_local[order].astype(np.int64)
    first = np.r_[True, ts[1:] != ts[:-1]]
    runstart = np.maximum.accumulate(np.where(first, np.arange(n), 0))
    occ = np.arange(n) - runstart
    kmax = int(occ.max())
    if kmax >= nchb:
        raise RuntimeError(
            f"target multiplicity {kmax + 1} exceeds chunk count {nchb}")
    for seed in range(16):
        h = ((ts * 2654435761 + seed * 97) % nchb).astype(np.int64)
        chunk_s = (occ + h) % nchb
        sizes = np.bincount(chunk_s, minlength=nchb)
        if sizes.max() <= m:
            break
    else:
        raise RuntimeError(f"chunk overflow: max {sizes.max()} > {m}")
    ord2 = np.argsort(chunk_s, kind="stable")
    starts = np.cumsum(np.r_[0, sizes[:-1]])
    pos_sorted = np.arange(n) - np.repeat(starts, sizes)
    pos_s = np.empty(n, np.int64)
    pos_s[ord2] = pos_sorted
    chunk = np.empty(n, np.int64)
    pos = np.empty(n, np.int64)
    chunk[order] = chunk_s
    pos[order] = pos_s
    return chunk, pos


def _prep_inputs(cfg, x, edge_index, W_self, W_neigh, bias):
    """Host-side sharding: partition edges by target owner, group by source
    bank, deal into duplicate-free chunks, pad, int16-encode, 16-partition
    wrap, replicate across the 8 Q7 groups."""
    x = np.ascontiguousarray(np.asarray(x, dtype=np.float32))
    ei = np.asarray(edge_index)
    row = ei[0].astype(np.int64)
    col = ei[1].astype(np.int64)
    wnT = np.ascontiguousarray(np.asarray(W_neigh, np.float32).T)
    wsT = np.ascontiguousarray(np.asarray(W_self, np.float32).T)
    bias_c = np.ascontiguousarray(
        np.asarray(bias, np.float32).reshape(cfg.D, 1))

    owner = col // cfg.W
    in_maps = []
    for c in range(cfg.NCORE):
        msk = owner == c
        r = row[msk]
        t = (col[msk] - c * cfg.W).astype(np.int64)
        b = r // cfg.BANK
        lo = r % cfg.BANK
        gbuf = np.zeros(cfg.TOK, np.int16)
        sbuf = np.full(cfg.TOK, cfg.W, np.int16)   # pads -> dead cell
        for bk in range(cfg.NBANK):
            sel = b == bk
            k = int(sel.sum())
            if k > cfg.BANK_CAP:
                raise RuntimeError(
                    f"bucket overflow: core {c} bank {bk} has {k} edges "
                    f"(capacity {cfg.BANK_CAP})")
            chunk, pos = _pack_bucket(t[sel], lo[sel], cfg.NCHB, cfg.M)
            slot = bk * cfg.BANK_CAP + chunk * cfg.M + pos
            gbuf[slot] = lo[sel].astype(np.int16)
            sbuf[slot] = t[sel].astype(np.int16)
        xw = np.zeros((cfg.D, cfg.WPAD), np.float32)
        xw[:, :cfg.W] = x[c * cfg.W:(c + 1) * cfg.W].T
        in_maps.append({
            "x": x,
            "xT": xw,
            "gidx": np.tile(np.ascontiguousarray(
                gbuf.reshape(-1, 16).T), (8, 1)),
            "sidx": np.tile(np.ascontiguousarray(
                sbuf.reshape(-1, 16).T), (8, 1)),
            "wnT": wnT,
            "wsT": wsT,
            "bias": bias_c,
        })
    return in_maps


_CACHED_NC = None


def _get_nc():
    global _CACHED_NC
    if _CACHED_NC is None:
        _CACHED_NC = _build(CFG)
    return _CACHED_NC


def kernel(x, edge_index, W_self, W_neigh, bias, _trace=False, _trace_kwargs=None):
    from concourse.bass_utils import run_bass_kernel_spmd

    cfg = CFG
    nc = _get_nc()
    in_maps = _prep_inputs(cfg, x, edge_index, W_self, W_neigh, bias)
    kw = {}
    if _trace:
        kw["trace"] = True
        if _trace_kwargs:
            kw.update(_trace_kwargs)
    res = run_bass_kernel_spmd(nc, in_maps, list(range(cfg.NCORE)), **kw)
    out = np.concatenate(
        [res.results[c]["outT"][:, :cfg.W].T for c in range(cfg.NCORE)], axis=0)
    out = np.ascontiguousarray(out, dtype=np.float32)
    if _trace:
        return out, res
    return out



# revision 2
# speedup vs baseline: 1.4879x; 1.4879x over previous
"""DimGraphConv (GNN mean-aggregation) on 8 Trainium2 cores — matmul segsum.

Math (per reference):
    out = x @ W_self.T + segment_mean(x[row], col) @ W_neigh.T + bias

Sharding: edges partitioned by TARGET owner (core c owns nodes
[c*12500, (c+1)*12500)), x replicated in HBM, weights replicated.

Phase 1 (the hot loop) avoids dma_scatter_add entirely.  The Q7 SWDGE
descriptor generation (2 cores per instruction, ~8 ns/descriptor) is the
real bottleneck of gather/scatter on TRN2, so the scatter's 225K
descriptors/core are replaced by PE matmuls:

  * Host sorts each core's edges by source bank (int16 gather index
    limit: 32767 rows -> 4 banks of 25000) and, within a bank, by target.
  * Targets are grouped in 98 windows of 128 slots.  Per (bank, window)
    the token capacity is max over the 8 cores (shared static schedule),
    shorter cores pad with slot=-1 tokens.
  * Device gathers M tokens/chunk (one descriptor per token, unchanged),
    converts to bf16, and for each 128-token tile builds a one-hot
    matrix S[i,j] = (slot[i] == j) with a DVE is_equal compare, then
    matmul(PSUM[window] += S^T @ G) performs the segment-sum.  Tiles
    straddling a window boundary issue a second compare+matmul (slots
    for the two windows are uploaded as separate slotA/slotB arrays;
    non-members are -1 and match nothing).  Full PSUM windows are added
    into an SBUF accumulator.
  * Duplicate targets inside a tile are summed by the matmul itself, so
    no duplicate-free chunk dealing is needed; pad tokens gather row 0
    with weight 0 everywhere.

In-degree is derived from edge_index on the host (bincount) and shipped
as a reciprocal table, so no degree accumulation happens on device.

Phase 2 per 128-target slot: mean = acc * recip, transpose via PE,
out^T = W_neigh @ mean^T + W_self @ x^T + bias (as before).
"""
import sys

sys.path.insert(0, "/opt/trn_rl_repo")

import numpy as np


# ---------------------------------------------------------------- config
class _Cfg:
    N = 100000          # nodes
    D = 64              # feature dim (256B rows = dma_gather elem)
    NCORE = 8
    W = 12500           # target window per core
    NBANK = 4           # source banks (int16 gather index limit)
    BANK = 25000        # rows per source bank
    M = 8448            # max tokens per gather chunk (66*128)
    REPEAT = 1          # timing-only: re-run the phase-1 loop this many times
    GRP = 7             # phase-2 slots per output DMA group

    @property
    def NSLOT(self):
        # slots of 128 target nodes (12544 >= 12500)
        return self.W // 128 + 1

    @property
    def NGRPOUT(self):
        return self.NSLOT // self.GRP

    @property
    def WPAD(self):
        return self.NSLOT * 128


CFG = _Cfg()


def _schedule(cfg, caps):
    """Static per-tile matmul schedule shared by all cores.

    caps: [NBANK, NSLOT] int64 token capacity per (bank, window) cell.
    Returns (bank_len, bank_base, TOK, NTILE, cell_lo, tile_entries,
    cell_list) where tile_entries[T] is an ordered list of
    (bank, window, which) and cell_list[(b, w)] is the ordered tile list.
    """
    bank_raw = caps.sum(axis=1)
    bank_len = ((bank_raw + 127) // 128) * 128
    bank_base = np.concatenate([[0], np.cumsum(bank_len)])[: cfg.NBANK]
    TOK = int(bank_len.sum())
    NTILE = TOK // 128

    starts = np.cumsum(caps, axis=1) - caps  # exclusive prefix per bank
    cell_lo = bank_base[:, None] + starts    # [NBANK, NSLOT]

    tile_entries = [[] for _ in range(NTILE)]
    cell_tiles = {}
    for b in range(cfg.NBANK):
        for w in range(cfg.NSLOT):
            cap = int(caps[b, w])
            if cap == 0:
                continue
            lo = int(cell_lo[b, w])
            hi = lo + cap
            t0, t1 = lo // 128, (hi - 1) // 128
            tiles = list(range(t0, t1 + 1))
            cell_tiles[(b, w)] = tiles
            for T in tiles:
                which = "B" if lo > T * 128 else "A"
                tile_entries[T].append((b, w, which))
    for T, ents in enumerate(tile_entries):
        assert len(ents) <= 2, f"tile {T} intersects {len(ents)} cells"
        if len(ents) == 2:
            assert ents[0][2] == "A" and ents[1][2] == "B", ents
    return bank_len, bank_base, TOK, NTILE, cell_lo, tile_entries, cell_tiles


def _build(cfg, caps):
    """Build + compile the SPMD Bass program for the given cell caps."""
    import concourse.tile as tile
    from concourse import bacc, mybir
    from concourse.masks import make_identity

    P = 128
    D = cfg.D
    MS = cfg.M // P
    f32 = mybir.dt.float32
    bf16 = mybir.dt.bfloat16

    (bank_len, bank_base, TOK, NTILE, cell_lo, tile_entries,
     cell_tiles) = _schedule(cfg, caps)

    # per-cell matmul counts -> first/last flags
    cell_n = {c: len(ts) for c, ts in cell_tiles.items()}

    nc = bacc.Bacc(None, target_bir_lowering=False, debug=False)
    x_d = nc.dram_tensor("x", [cfg.N, D], f32, kind="ExternalInput")
    xT_d = nc.dram_tensor("xT", [D, cfg.WPAD], f32, kind="ExternalInput")
    gidx_d = nc.dram_tensor("gidx", [128, TOK // 16], mybir.dt.int16,
                            kind="ExternalInput")
    slotA_d = nc.dram_tensor("slotA", [128, NTILE], f32, kind="ExternalInput")
    slotB_d = nc.dram_tensor("slotB", [128, NTILE], f32, kind="ExternalInput")
    iota_d = nc.dram_tensor("iota", [128, 128], f32, kind="ExternalInput")
    recip_d = nc.dram_tensor("recip", [128, cfg.NSLOT], f32,
                             kind="ExternalInput")
    wnT_d = nc.dram_tensor("wnT", [D, D], f32, kind="ExternalInput")
    wsT_d = nc.dram_tensor("wsT", [D, D], f32, kind="ExternalInput")
    bias_d = nc.dram_tensor("bias", [D, 1], f32, kind="ExternalInput")
    outT_d = nc.dram_tensor("outT", [D, cfg.WPAD], f32, kind="ExternalOutput")

    with tile.TileContext(nc) as tc:
        with (
            tc.tile_pool(name="acc", bufs=1) as accp,
            tc.tile_pool(name="io", bufs=3) as iop,
            tc.tile_pool(name="sel", bufs=4) as selp,
            tc.tile_pool(name="p2", bufs=3) as p2p,
            tc.tile_pool(name="psum", bufs=4, space="PSUM") as psp,
            tc.tile_pool(name="psum2", bufs=2, space="PSUM") as psp2,
        ):
            gidx_t = accp.tile([128, TOK // 16], mybir.dt.int16)
            slotA_t = accp.tile([128, NTILE], f32)
            slotB_t = accp.tile([128, NTILE], f32)
            iota_t = accp.tile([128, 128], f32)
            recip_t = accp.tile([128, cfg.NSLOT], f32)
            nc.sync.dma_start(gidx_t[:], gidx_d[:])
            nc.sync.dma_start(slotA_t[:], slotA_d[:])
            nc.sync.dma_start(slotB_t[:], slotB_d[:])
            nc.sync.dma_start(iota_t[:], iota_d[:])
            nc.sync.dma_start(recip_t[:], recip_d[:])

            acc_t = accp.tile([P, cfg.NSLOT * D], f32)
            nc.vector.memset(acc_t[:], 0.0)

            # ---- phase 1: gather + matmul segment-sum
            for _rep in range(cfg.REPEAT):
                psums = {}
                done = {c: 0 for c in cell_tiles}
                T = 0
                for b in range(cfg.NBANK):
                    blen = int(bank_len[b])
                    chunks = [cfg.M] * (blen // cfg.M)
                    if blen % cfg.M:
                        chunks.append(blen % cfg.M)
                    coff = 0
                    for csz in chunks:
                        tok0 = int(bank_base[b]) + coff
                        coff += csz
                        cms = csz // P
                        sl = slice(tok0 // 16, (tok0 + csz) // 16)
                        gath_t = iop.tile([P, cms * D], f32, name="gath")
                        nc.gpsimd.dma_gather(
                            out_ap=gath_t[:].rearrange("p (m e) -> p m e",
                                                       e=D),
                            in_ap=x_d[b * cfg.BANK:(b + 1) * cfg.BANK, :],
                            idxs_ap=gidx_t[:, sl],
                            num_idxs=csz,
                            num_idxs_reg=csz,
                            elem_size=D,
                            single_packet=False,
                        )
                        gbf_t = iop.tile([P, cms * D], bf16, name="gbf")
                        nc.vector.tensor_copy(gbf_t[:], gath_t[:])
                        gbf3 = gbf_t[:].rearrange("p (m e) -> p m e", e=D)
                        for m in range(cms):
                            for (bb, w, which) in tile_entries[T]:
                                cell = (bb, w)
                                slot_t = slotA_t if which == "A" else slotB_t
                                s_t = selp.tile([P, P], bf16)
                                nc.vector.tensor_scalar(
                                    s_t[:], iota_t[:], slot_t[:, T:T + 1],
                                    None, mybir.AluOpType.is_equal)
                                if cell not in psums:
                                    psums[cell] = psp.tile(
                                        [P, D], f32, name="cellps")
                                first = done[cell] == 0
                                last = done[cell] == cell_n[cell] - 1
                                nc.tensor.matmul(psums[cell][:], lhsT=s_t[:],
                                                 rhs=gbf3[:, m, :],
                                                 start=first, stop=last)
                                done[cell] += 1
                                if last:
                                    acc_sl = acc_t[:, w * D:(w + 1) * D]
                                    nc.vector.tensor_tensor(
                                        acc_sl, acc_sl, psums[cell][:],
                                        mybir.AluOpType.add)
                                    del psums[cell]
                            T += 1

            # ---- phase 2: mean, two GEMMs, bias; output transposed
            ident_t = p2p.tile([P, P], f32)
            make_identity(nc, ident_t[:])
            wnT_t = p2p.tile([D, D], f32)
            wsT_t = p2p.tile([D, D], f32)
            bias_t = p2p.tile([D, 1], f32)
            nc.sync.dma_start(wnT_t[:], wnT_d[:])
            nc.sync.dma_start(wsT_t[:], wsT_d[:])
            nc.sync.dma_start(bias_t[:], bias_d[:])

            gw = cfg.GRP * P
            for og in range(cfg.NGRPOUT):
                xTg_t = p2p.tile([D, gw], f32)
                outg_t = p2p.tile([D, gw], f32)
                nc.sync.dma_start(xTg_t[:], xT_d[:, og * gw:(og + 1) * gw])
                for k in range(cfg.GRP):
                    s = og * cfg.GRP + k
                    mean_t = p2p.tile([P, D], f32)
                    nc.vector.tensor_scalar_mul(
                        mean_t[:], acc_t[:, s * D:(s + 1) * D],
                        recip_t[:, s:s + 1])
                    psT_t = psp2.tile([D, P], f32)
                    nc.tensor.transpose(psT_t[:], mean_t[:], ident_t[:])
                    meanT_t = p2p.tile([D, P], f32)
                    nc.vector.tensor_copy(meanT_t[:], psT_t[:])
                    po_t = psp2.tile([D, P], f32)
                    nc.tensor.matmul(po_t[:], lhsT=wnT_t[:], rhs=meanT_t[:],
                                     start=True, stop=False)
                    nc.tensor.matmul(po_t[:], lhsT=wsT_t[:],
                                     rhs=xTg_t[:, k * P:(k + 1) * P],
                                     start=False, stop=True)
                    nc.vector.tensor_scalar_add(outg_t[:, k * P:(k + 1) * P],
                                                po_t[:], bias_t[:, 0:1])
                nc.sync.dma_start(outT_d[:, og * gw:(og + 1) * gw], outg_t[:])

    nc.compile()
    return nc


def _edge_layout(cfg, edge_index):
    """Host-side: per-core token layout + caps + degree reciprocals."""
    ei = np.asarray(edge_index)
    row = ei[0].astype(np.int64)
    col = ei[1].astype(np.int64)
    owner = col // cfg.W

    cores = []
    counts = np.zeros((cfg.NCORE, cfg.NBANK, cfg.NSLOT), np.int64)
    for c in range(cfg.NCORE):
        msk = owner == c
        r = row[msk]
        t = col[msk] - c * cfg.W
        b = r // cfg.BANK
        lo = r % cfg.BANK
        w = t // 128
        counts[c] = np.bincount(
            b * cfg.NSLOT + w, minlength=cfg.NBANK * cfg.NSLOT
        ).reshape(cfg.NBANK, cfg.NSLOT)
        deg = np.bincount(t, minlength=cfg.WPAD).astype(np.float32)
        recip = 1.0 / np.maximum(deg, 1.0)
        cores.append((r, t, b, lo, recip))
    caps = counts.max(axis=0)
    return cores, caps


def _prep_inputs(cfg, caps, sched, cores, x, W_self, W_neigh, bias):
    (bank_len, bank_base, TOK, NTILE, cell_lo, tile_entries,
     cell_tiles) = sched
    x = np.ascontiguousarray(np.asarray(x, dtype=np.float32))
    wnT = np.ascontiguousarray(np.asarray(W_neigh, np.float32).T)
    wsT = np.ascontiguousarray(np.asarray(W_self, np.float32).T)
    bias_c = np.ascontiguousarray(
        np.asarray(bias, np.float32).reshape(cfg.D, 1))
    iota = np.ascontiguousarray(
        np.tile(np.arange(128, dtype=np.float32), (128, 1)))

    in_maps = []
    for c in range(cfg.NCORE):
        r, t, b, lo, recip = cores[c]
        w = t // 128
        slot = t % 128
        # position within cell: stable rank among same (b, w)
        key = b * cfg.NSLOT + w
        order = np.argsort(key, kind="stable")
        k_sorted = key[order]
        first = np.r_[True, k_sorted[1:] != k_sorted[:-1]]
        runstart = np.maximum.accumulate(
            np.where(first, np.arange(len(k_sorted)), 0))
        occ_sorted = np.arange(len(k_sorted)) - runstart
        occ = np.empty(len(k_sorted), np.int64)
        occ[order] = occ_sorted
        pos = cell_lo[b, w] + occ
        if (occ >= caps[b, w]).any():
            raise RuntimeError("cell capacity overflow vs compiled schedule")

        gidx = np.zeros(TOK, np.int16)
        slotA = np.full(TOK, -1.0, np.float32)
        slotB = np.full(TOK, -1.0, np.float32)
        gidx[pos] = lo.astype(np.int16)
        tile_of = pos // 128
        is_b = cell_lo[b, w] > tile_of * 128
        slotA[pos[~is_b]] = slot[~is_b]
        slotB[pos[is_b]] = slot[is_b]

        xw = np.zeros((cfg.D, cfg.WPAD), np.float32)
        xw[:, :cfg.W] = x[c * cfg.W:(c + 1) * cfg.W].T
        in_maps.append({
            "x": x,
            "xT": xw,
            "gidx": np.tile(np.ascontiguousarray(
                gidx.reshape(-1, 16).T), (8, 1)),
            "slotA": np.ascontiguousarray(slotA.reshape(NTILE, 128).T),
            "slotB": np.ascontiguousarray(slotB.reshape(NTILE, 128).T),
            "iota": iota,
            "recip": np.ascontiguousarray(
                recip.reshape(cfg.NSLOT, 128).T),
            "wnT": wnT,
            "wsT": wsT,
            "bias": bias_c,
        })
    return in_maps


_CACHED = None  # (caps, sched, nc)


def _get_nc(cfg, edge_index):
    global _CACHED
    from concourse import bass  # noqa: F401  (ensure repo import works)
    cores, caps = _edge_layout(cfg, edge_index)
    if _CACHED is not None:
        caps0, sched0, nc0 = _CACHED
        if (caps <= caps0).all():
            return cores, caps0, sched0, nc0
    sched = _schedule(cfg, caps)
    nc = _build(cfg, caps)
    _CACHED = (caps, sched, nc)
    return cores, caps, sched, nc


def kernel(x, edge_index, W_self, W_neigh, bias, _trace=False,
           _trace_kwargs=None):
    from concourse.bass_utils import run_bass_kernel_spmd

    cfg = CFG
    cores, caps, sched, nc = _get_nc(cfg, edge_index)
    in_maps = _prep_inputs(cfg, caps, sched, cores, x, W_self, W_neigh, bias)
    kw = {}
    if _trace:
        kw["trace"] = True
        if _trace_kwargs:
            kw.update(_trace_kwargs)
    res = run_bass_kernel_spmd(nc, in_maps, list(range(cfg.NCORE)), **kw)
    out = np.concatenate(
        [res.results[c]["outT"][:, :cfg.W].T for c in range(cfg.NCORE)],
        axis=0)
    out = np.ascontiguousarray(out, dtype=np.float32)
    if _trace:
        return out, res
    return out
